# revision 1
# baseline (speedup 1.0000x reference)
"""Trainium2 Bass kernel for an 8-layer dense MLP (nn_FCN).

Reference computation (fp32):
    y0 = x                                  [4096, 2048]
    y_{l+1} = relu((y_l @ W_l.T) / sqrt(2048))   W: [8, 2048, 2048]
    out = y_8 @ beta / 2048                 beta: [2048, 1024] -> out [4096, 1024]

Strategy: data-parallel over batch across 8 NeuronCores (512 rows each);
weights/beta replicated. On-chip layout is channel-major ("transposed"):
activations live in SBUF as [128(part)=ch, 16(k-tile), 512(batch)], so each
layer is out[o, b] = sum_i WT[i, o] * Y[i, b] with the contraction dim on
partitions, and layer outputs land back in the same channel-major layout —
no transposes anywhere on device. W is pre-transposed once on the host
(WT[l] = W[l].T, contiguous); beta is already [h, out] = lhsT layout.

Matmuls run as float32r (TF32-like, 11-bit mantissa, full PE issue rate at
N=512) accumulating fp32 in PSUM; the ReLU epilogue runs on the scalar
engine reading PSUM and writing the next layer's fp32r activation tile.
"""

import math
from contextlib import ExitStack

import numpy as np

P = 128
H = 2048
OUT = 1024
B_TOTAL = 4096
N_CORES = 8
B = B_TOTAL // N_CORES  # 512 batch rows per core
L = 8
KI = H // P  # 16 contraction tiles per matmul
O_CHUNK = 512  # output channels per weight DMA chunk
SCALE = 1.0 / math.sqrt(H)
OUT_SCALE = 1.0 / H

_BUILD_CACHE = {}


def _build(repeat=1, loop=0):
    """loop>0 wraps `repeat` forward passes in an on-device For_i loop of
    `loop` iterations (timing tool only; grading path uses repeat=1, loop=0)."""
    key = (repeat, loop)
    if key in _BUILD_CACHE:
        return _BUILD_CACHE[key]

    import concourse.mybir as mybir
    import concourse.tile as tile
    from concourse import bacc

    f32 = mybir.dt.float32
    f32r = mybir.dt.float32r

    nc = bacc.Bacc("TRN2", target_bir_lowering=False, debug=False)
    xt_d = nc.dram_tensor("xt", [H, B], f32r, kind="ExternalInput").ap()
    wt_d = nc.dram_tensor("wt", [L, H, H], f32r, kind="ExternalInput").ap()
    beta_d = nc.dram_tensor("beta", [H, OUT], f32r, kind="ExternalInput").ap()
    out_d = nc.dram_tensor("out", [OUT, B], f32, kind="ExternalOutput").ap()

    xt_t = xt_d.rearrange("(k p) b -> p k b", p=P)
    out_t = out_d.rearrange("(k p) b -> p k b", p=P)
    beta_t = beta_d.rearrange("(k p) o -> p k o", p=P)

    with tile.TileContext(nc) as tc, ExitStack() as ctx:
        ypool = ctx.enter_context(tc.tile_pool(name="y", bufs=2))
        wpool = ctx.enter_context(tc.tile_pool(name="w", bufs=3))
        opool = ctx.enter_context(tc.tile_pool(name="o", bufs=4))
        pspool = ctx.enter_context(tc.tile_pool(name="ps", bufs=8, space="PSUM"))

        # Warm the PE (HAM clock-gate releases after ~3.4us of sustained
        # activity) with dummy matmuls on zeroed tiles while the first x and
        # weight DMAs are in flight; the array hits 2.4GHz before real work.
        warm_w = opool.tile([P, P], f32r, tag="warmw")
        warm_y = opool.tile([P, B], f32r, tag="warmy")
        nc.sync.dma_start(warm_w[:], beta_t[:, 0, 0:P])
        nc.sync.dma_start(warm_y[:], xt_t[:, 0, :])
        # 8 cold-rate dummies ~= 3.4us: exactly one HAM window, finishing as
        # the first (split) weight k-group lands, so real MMs start warm.
        for _d in range(8):
            ps = pspool.tile([P, B], f32, tag="ps")
            nc.tensor.matmul(
                ps[:], lhsT=warm_w[:], rhs=warm_y[:], start=True, stop=True
            )

        if loop:
            loop_cm = tc.For_i(0, loop, 1)
            loop_cm.__enter__()

        for _ in range(repeat):
            y_cur = ypool.tile([P, KI, B], f32r, tag="y")
            for kg in range(4):
                nc.sync.dma_start(
                    y_cur[:, kg * 4 : (kg + 1) * 4, :],
                    xt_t[:, kg * 4 : (kg + 1) * 4, :],
                )

            for layer in range(L):
                wt_l = wt_d[layer].rearrange("(k p) o -> p k o", p=P)
                y_next = ypool.tile([P, KI, B], f32r, tag="y")
                for oc in range(H // O_CHUNK):
                    w_sb = wpool.tile([P, KI, O_CHUNK], f32r, tag="w")
                    if layer == 0 and oc == 0:
                        # split the very first chunk by k-groups so layer-0
                        # matmuls start after ~1MB instead of the full 4MB
                        for kg in range(4):
                            nc.sync.dma_start(
                                w_sb[:, kg * 4 : (kg + 1) * 4, :],
                                wt_l[:, kg * 4 : (kg + 1) * 4, 0:O_CHUNK],
                            )
                    else:
                        nc.sync.dma_start(
                            w_sb[:], wt_l[:, :, oc * O_CHUNK : (oc + 1) * O_CHUNK]
                        )
                    for os_ in range(O_CHUNK // P):
                        ps = pspool.tile([P, B], f32, tag="ps")
                        for ki in range(KI):
                            nc.tensor.matmul(
                                ps[:],
                                lhsT=w_sb[:, ki, os_ * P : (os_ + 1) * P],
                                rhs=y_cur[:, ki, :],
                                start=(ki == 0),
                                stop=(ki == KI - 1),
                            )
                        ot = oc * (O_CHUNK // P) + os_
                        nc.scalar.activation(
                            y_next[:, ot, :],
                            ps[:],
                            mybir.ActivationFunctionType.Relu,
                            scale=SCALE,
                        )
                y_cur = y_next

            # readout: out[o, b] = sum_h beta[h, o] * y[h, b], scaled by 1/H
            for oc in range(OUT // O_CHUNK):
                b_sb = wpool.tile([P, KI, O_CHUNK], f32r, tag="w")
                nc.sync.dma_start(
                    b_sb[:], beta_t[:, :, oc * O_CHUNK : (oc + 1) * O_CHUNK]
                )
                for os_ in range(O_CHUNK // P):
                    ps = pspool.tile([P, B], f32, tag="ps")
                    for ki in range(KI):
                        nc.tensor.matmul(
                            ps[:],
                            lhsT=b_sb[:, ki, os_ * P : (os_ + 1) * P],
                            rhs=y_cur[:, ki, :],
                            start=(ki == 0),
                            stop=(ki == KI - 1),
                        )
                    o_sb = opool.tile([P, B], f32, tag="o")
                    nc.scalar.activation(
                        o_sb[:],
                        ps[:],
                        mybir.ActivationFunctionType.Copy,
                        scale=OUT_SCALE,
                    )
                    nc.sync.dma_start(
                        out_t[:, oc * (O_CHUNK // P) + os_, :], o_sb[:]
                    )

        if loop:
            loop_cm.__exit__(None, None, None)

    nc.compile()
    _BUILD_CACHE[key] = nc
    return nc


def _prep_in_maps(x, W, beta):
    x = np.asarray(x, dtype=np.float32)
    W = np.asarray(W, dtype=np.float32)
    beta = np.asarray(beta, dtype=np.float32)
    xt = np.ascontiguousarray(x.T)  # [H, B_TOTAL]
    wt = np.ascontiguousarray(W.transpose(0, 2, 1))  # [L, H(i), H(o)]
    beta = np.ascontiguousarray(beta)
    return [
        {"xt": np.ascontiguousarray(xt[:, c * B : (c + 1) * B]), "wt": wt, "beta": beta}
        for c in range(N_CORES)
    ]


def kernel(x, W, beta):
    from concourse.bass_utils import run_bass_kernel_spmd

    nc = _build()
    in_maps = _prep_in_maps(x, W, beta)
    res = run_bass_kernel_spmd(nc, in_maps, core_ids=list(range(N_CORES)))
    outs = [r["out"] for r in res.results]  # each [OUT, B] channel-major
    return np.concatenate([o.T for o in outs], axis=0).astype(np.float32)



# revision 2
# speedup vs baseline: 1.0406x; 1.0406x over previous
"""Trainium2 Bass kernel for an 8-layer dense MLP (nn_FCN).

Reference computation (fp32):
    y0 = x                                  [4096, 2048]
    y_{l+1} = relu((y_l @ W_l.T) / sqrt(2048))   W: [8, 2048, 2048]
    out = y_8 @ beta / 2048                 beta: [2048, 1024] -> out [4096, 1024]

Strategy: data-parallel over batch across 8 NeuronCores (512 rows each);
weights/beta replicated. On-chip layout is channel-major ("transposed"):
activations live in SBUF as [128(part)=ch, 16(k-tile), 512(batch)], so each
layer is out[o, b] = sum_i WT[i, o] * Y[i, b] with the contraction dim on
partitions, and layer outputs land back in the same channel-major layout —
no transposes anywhere on device.

Numerics: weights/beta/activations are bfloat16 (halves the ~150MB/core of
HBM traffic that made the fp32r version HBM-contended across 8 cores);
matmuls run at the same 1 row/cycle as fp32r, accumulating fp32 in PSUM.
The ReLU epilogue runs on the scalar engine reading PSUM fp32 and writing
the next layer's bf16 activation tile; the readout epilogue writes fp32.
Measured end-to-end relative error ~9e-3 (tolerance 2e-2).

Weights are repacked on the host into per-(layer, chunk) blocks
[L, OC, 128(p), 16(k)*512(o)] so every weight DMA is one fully-contiguous
16KB descriptor per partition (vs 2KB strided rows of W.T).
"""

import math
from contextlib import ExitStack

import numpy as np

P = 128
H = 2048
OUT = 1024
B_TOTAL = 4096
N_CORES = 8
B = B_TOTAL // N_CORES  # 512 batch rows per core
L = 8
KI = H // P  # 16 contraction tiles per matmul
O_CHUNK = 512  # output channels per weight DMA chunk
OC = H // O_CHUNK  # 4 weight chunks per layer
OC_BETA = OUT // O_CHUNK  # 2 chunks for beta
SCALE = 1.0 / math.sqrt(H)
OUT_SCALE = 1.0 / H

_BUILD_CACHE = {}


def _build(repeat=1, loop=0):
    """loop>0 wraps `repeat` forward passes in an on-device For_i loop of
    `loop` iterations (timing tool only; grading path uses repeat=1, loop=0)."""
    key = (repeat, loop)
    if key in _BUILD_CACHE:
        return _BUILD_CACHE[key]

    import concourse.mybir as mybir
    import concourse.tile as tile
    from concourse import bacc

    f32 = mybir.dt.float32
    bf16 = mybir.dt.bfloat16

    nc = bacc.Bacc("TRN2", target_bir_lowering=False, debug=False)
    xt_d = nc.dram_tensor("xt", [P, KI, B], bf16, kind="ExternalInput").ap()
    wt_d = nc.dram_tensor(
        "wt", [L, OC, P, KI, O_CHUNK], bf16, kind="ExternalInput"
    ).ap()
    beta_d = nc.dram_tensor(
        "beta", [OC_BETA, P, KI, O_CHUNK], bf16, kind="ExternalInput"
    ).ap()
    out_d = nc.dram_tensor("out", [OUT, B], f32, kind="ExternalOutput").ap()

    out_t = out_d.rearrange("(k p) b -> p k b", p=P)

    with tile.TileContext(nc) as tc, ExitStack() as ctx:
        ypool = ctx.enter_context(tc.tile_pool(name="y", bufs=2))
        wpool = ctx.enter_context(tc.tile_pool(name="w", bufs=3))
        opool = ctx.enter_context(tc.tile_pool(name="o", bufs=4))
        pspool = ctx.enter_context(tc.tile_pool(name="ps", bufs=8, space="PSUM"))

        # Warm the PE (HAM clock-gate releases after ~3.4us of sustained
        # activity) with dummy matmuls on tiles loaded by the first DMAs;
        # the array hits 2.4GHz before real work.
        warm_w = opool.tile([P, P], bf16, tag="warmw")
        warm_y = opool.tile([P, B], bf16, tag="warmy")
        nc.sync.dma_start(warm_w[:], beta_d[0, :, 0, 0:P])
        nc.sync.dma_start(warm_y[:], xt_d[:, 0, :])
        # 8 cold-rate dummies ~= 3.4us: exactly one HAM window, finishing as
        # the first (split) weight k-group lands, so real MMs start warm.
        for _d in range(8):
            ps = pspool.tile([P, B], f32, tag="ps")
            nc.tensor.matmul(
                ps[:], lhsT=warm_w[:], rhs=warm_y[:], start=True, stop=True
            )

        if loop:
            loop_cm = tc.For_i(0, loop, 1)
            loop_cm.__enter__()

        for _ in range(repeat):
            y_cur = ypool.tile([P, KI, B], bf16, tag="y")
            for kg in range(4):
                nc.sync.dma_start(
                    y_cur[:, kg * 4 : (kg + 1) * 4, :],
                    xt_d[:, kg * 4 : (kg + 1) * 4, :],
                )

            for layer in range(L):
                y_next = ypool.tile([P, KI, B], bf16, tag="y")
                for oc in range(OC):
                    w_sb = wpool.tile([P, KI, O_CHUNK], bf16, tag="w")
                    if layer == 0 and oc == 0:
                        # split the very first chunk by k-groups so layer-0
                        # matmuls start after ~0.5MB instead of the full 2MB
                        for kg in range(4):
                            nc.sync.dma_start(
                                w_sb[:, kg * 4 : (kg + 1) * 4, :],
                                wt_d[layer, oc, :, kg * 4 : (kg + 1) * 4, :],
                            )
                    else:
                        nc.sync.dma_start(w_sb[:], wt_d[layer, oc])
                    for os_ in range(O_CHUNK // P):
                        ps = pspool.tile([P, B], f32, tag="ps")
                        for ki in range(KI):
                            nc.tensor.matmul(
                                ps[:],
                                lhsT=w_sb[:, ki, os_ * P : (os_ + 1) * P],
                                rhs=y_cur[:, ki, :],
                                start=(ki == 0),
                                stop=(ki == KI - 1),
                            )
                        ot = oc * (O_CHUNK // P) + os_
                        nc.scalar.activation(
                            y_next[:, ot, :],
                            ps[:],
                            mybir.ActivationFunctionType.Relu,
                            scale=SCALE,
                        )
                y_cur = y_next

            # readout: out[o, b] = sum_h beta[h, o] * y[h, b], scaled by 1/H
            for oc in range(OC_BETA):
                b_sb = wpool.tile([P, KI, O_CHUNK], bf16, tag="w")
                nc.sync.dma_start(b_sb[:], beta_d[oc])
                for os_ in range(O_CHUNK // P):
                    ps = pspool.tile([P, B], f32, tag="ps")
                    for ki in range(KI):
                        nc.tensor.matmul(
                            ps[:],
                            lhsT=b_sb[:, ki, os_ * P : (os_ + 1) * P],
                            rhs=y_cur[:, ki, :],
                            start=(ki == 0),
                            stop=(ki == KI - 1),
                        )
                    o_sb = opool.tile([P, B], f32, tag="o")
                    nc.scalar.activation(
                        o_sb[:],
                        ps[:],
                        mybir.ActivationFunctionType.Copy,
                        scale=OUT_SCALE,
                    )
                    nc.sync.dma_start(
                        out_t[:, oc * (O_CHUNK // P) + os_, :], o_sb[:]
                    )

        if loop:
            loop_cm.__exit__(None, None, None)

    nc.compile()
    _BUILD_CACHE[key] = nc
    return nc


def _prep_in_maps(x, W, beta):
    import ml_dtypes

    bf16 = ml_dtypes.bfloat16
    x = np.asarray(x, dtype=np.float32)
    W = np.asarray(W, dtype=np.float32)
    beta = np.asarray(beta, dtype=np.float32)

    # wt[l, oc, p, k, o] = W[l, oc*512+o, k*128+p]  (lhsT blocks, packed)
    wt = np.ascontiguousarray(
        W.reshape(L, OC, O_CHUNK, KI, P).transpose(0, 1, 4, 3, 2)
    ).astype(bf16)
    # bt[oc, p, k, o] = beta[k*128+p, oc*512+o]
    bt = np.ascontiguousarray(
        beta.reshape(KI, P, OC_BETA, O_CHUNK).transpose(2, 1, 0, 3)
    ).astype(bf16)

    in_maps = []
    for c in range(N_CORES):
        xc = x[c * B : (c + 1) * B, :]  # [B, H]
        # xt[p, k, b] = xc[b, k*128+p]
        xt = np.ascontiguousarray(xc.reshape(B, KI, P).transpose(2, 1, 0)).astype(
            bf16
        )
        in_maps.append({"xt": xt, "wt": wt, "beta": bt})
    return in_maps


def kernel(x, W, beta):
    from concourse.bass_utils import run_bass_kernel_spmd

    nc = _build()
    in_maps = _prep_in_maps(x, W, beta)
    res = run_bass_kernel_spmd(nc, in_maps, core_ids=list(range(N_CORES)))
    outs = [r["out"] for r in res.results]  # each [OUT, B] channel-major
    return np.concatenate([o.T for o in outs], axis=0).astype(np.float32)
